# revision 29
# baseline (speedup 1.0000x reference)
"""BlockEqLinear kernel for Trainium2 (8 NeuronCores, SPMD data-parallel over batch).

Math (reference):
    x: [4096, 4096] viewed as [B=4096, K=8, H=512]
    A, B: [G=4, H, H]
    out[b, g, k, :] = x_k[b] @ (A_g - B_g)^T + S[b] @ B_g^T,  S = sum_k x_k
    returned as [B, G*K*H] = [4096, 16384]

Strategy (measured ~146 us HW time on 8 cores; PE-stream floor ~124 us):
  - Shard batch across 8 cores (512 rows each); weights replicated.
  - Host-side layout prep only (transposes/packing + the trivial S
    reduction): all inputs packed into ONE partition-major DRAM tensor
    per core so a few large column-chunk DMAs feed a single resident
    SBUF tile; contraction dim (h) is the partition dim on chip.
  - Matmul operands in bf16 (PSUM accumulates fp32): same PE rate as
    fp32r but half the input DMA and fast weight loads; measured
    back-to-back matmul interval 216 ns = the N=512 streaming floor.
  - Dummy warm-up matmuls run during the input-DMA wait so the PE HAM
    clock gate is at 8/8 (2.4 GHz) when the real stream starts.
  - Phase A: tsum[bt,g] = S-tile @ B_g^T (g-major; first group needs
    only the first 0.25 MB chunk), evicted to SBUF by ScalarE.
  - Phase B: k-outer so x^T streams just-in-time one k-slice at a time:
    for k: for bt: for g: 4 matmuls -> PSUM; DVE eviction fuses the
    +tsum add and packs 4 g-slices into one staging tile; one output
    DMA per (k, bt) on the second HWDGE queue (ScalarE).
  - Output written as bf16 (halves write traffic -> PE-bound overall),
    upcast to fp32 on host. L2 rel err ~3.2e-3, absmax ~0.5% of scale.
"""

import numpy as np

import concourse.mybir as mybir
import concourse.tile as tile
from concourse import bacc
from concourse.bass_utils import run_bass_kernel_spmd
from contextlib import ExitStack

G, K, H = 4, 8, 512
B_TOTAL = 4096
NCORES = 8
BS = B_TOTAL // NCORES  # 512 batch rows per core
P = 128                 # partition dim
HC = H // P             # 4 contraction chunks per 512-dim h
NBT = BS // P           # 4 b-tiles per core

F32 = mybir.dt.float32
F32R = mybir.dt.float32r
BF16 = mybir.dt.bfloat16

OUT_BF16 = True         # write y as bf16 on device, upcast on host
MM_BF16 = True          # all matmul operands (x^T, S^T, weights) in bf16:
                        # halves input DMA, enables fast weight load
N_WARMUP = 9            # dummy PE matmuls to warm HAM during input DMA

_CACHE = {}


def _build():
    out_dt = BF16 if OUT_BF16 else F32

    nc = bacc.Bacc(
        "TRN2", target_bir_lowering=False, debug=False, num_devices=NCORES
    )

    # All inputs packed host-side into ONE partition-major tensor so a
    # few large column-chunk DMAs (one descriptor-gen each) feed SBUF.
    # Column layout (all per-partition-row contiguous in DRAM):
    #   [0, 2560):      for hc: [ st(bt0,hc) 128c | btw(g0,hc) 512c ]
    #                   (interleaved so the first sum matmul needs only
    #                   the first 640 cols)
    #   [2560, 4096):   st[bt1..3]   (bt, hc, b128)
    #   [4096, 10240):  btw[g1..3]   (g, hc, p)
    #   [10240, 18432): dtw          (g, hc, p)
    #   [18432, 34816): xt           (bt, k, hc, b128) — bt-major
    x_dt = BF16 if MM_BF16 else F32R
    NCOL_ST = HC * BS
    NCOL_W = G * HC * H
    NCOL_X = K * HC * BS
    NCOL = NCOL_ST + 2 * NCOL_W + NCOL_X
    inp = nc.dram_tensor("inp", [P, NCOL], x_dt, kind="ExternalInput")
    # y_dev[bt, k, p, g*512 + pp] = out[bt*128 + p, g, k, pp]
    y = nc.dram_tensor("y", [NBT, K, P, G * H], out_dt, kind="ExternalOutput")

    with tile.TileContext(nc) as tc, ExitStack() as ctx:
        wpool = ctx.enter_context(tc.tile_pool(name="w", bufs=1))
        xpool = ctx.enter_context(tc.tile_pool(name="x", bufs=1))
        tsump = ctx.enter_context(tc.tile_pool(name="tsum", bufs=1))
        opool = ctx.enter_context(tc.tile_pool(name="o", bufs=8))
        psd = ctx.enter_context(tc.tile_pool(name="psd", bufs=6, space="PSUM"))
        pss = ctx.enter_context(tc.tile_pool(name="pss", bufs=2, space="PSUM"))

        # PE warm-up scratch: zeroed tile for dummy matmuls (below) that
        # run while the first input DMAs are in flight, so HAM is at
        # K=8/8 (2.4 GHz) when the real matmul stream starts.
        scratch = wpool.tile([P, H], BF16)
        nc.gpsimd.memset(scratch[:], 0.0)

        # One SBUF-resident input tile; chunked column DMAs in
        # consumption order. chunk0 is split per-hc so the first sum
        # matmul can start after only 640 cols (0.16 MB).
        in_sb = xpool.tile([P, NCOL], x_dt)
        O_DT = NCOL_ST + NCOL_W
        O_XT = NCOL_ST + 2 * NCOL_W
        XBT = K * HC * P                              # x cols per b-tile
        chunks = [(hc * 640, (hc + 1) * 640) for hc in range(HC)]
        W1 = HC * H                                   # cols per weight g
        chunks += [
            (4096 + 0 * W1, 4096 + 1 * W1),           # btw[g1]
            (4096 + 1 * W1, 4096 + 2 * W1),           # btw[g2]
            (4096 + 2 * W1, 4096 + 3 * W1),           # btw[g3]
            (O_DT + 0 * W1, O_DT + 1 * W1),           # dtw[g0]
            (O_DT + 1 * W1, O_DT + 2 * W1),           # dtw[g1]
            (O_XT, O_XT + XBT // 2),                  # xt[bt0, k0..3]
            (O_DT + 2 * W1, O_DT + 3 * W1),           # dtw[g2]
            (O_DT + 3 * W1, O_DT + 4 * W1),           # dtw[g3]
            (O_XT + XBT // 2, O_XT + XBT),            # xt[bt0, k4..7]
            (2560, 4096),                             # st[bt1..3] (needed at bt1)
        ]
        for bt in range(1, NBT):
            chunks.append((O_XT + bt * XBT, O_XT + (bt + 1) * XBT))
        for c0, c1 in chunks:
            nc.sync.dma_start(in_sb[:, c0:c1], inp[:, c0:c1])

        def st_slice(hc, bt):
            if bt == 0:
                c = hc * 640
            else:
                c = 2560 + ((bt - 1) * HC + hc) * P
            return in_sb[:, c : c + P]

        def btw_slice(g, hc):
            if g == 0:
                c = hc * 640 + P
            else:
                c = 4096 + ((g - 1) * HC + hc) * H
            return in_sb[:, c : c + H]

        def xt_slice(bt, k, hc):
            c = O_XT + ((bt * K + k) * HC + hc) * P
            return in_sb[:, c : c + P]

        dt_sb = in_sb[:, O_DT : O_DT + NCOL_W]

        # Dummy warm-up matmuls (PE program order puts these before the
        # real stream; they execute during the input-DMA wait).
        warm_ps = pss.tile([P, H], F32, tag="ps")
        for i in range(N_WARMUP):
            nc.tensor.matmul(
                warm_ps[:],
                scratch[:, :P],
                scratch[:],
                start=True,
                stop=True,
            )

        # bt-outer: per b-tile, first the 4 sum groups (tsum[g]), then
        # the k-loop of diag matmuls with fused +tsum eviction. Each
        # segment's data needs are small, so the stream starts early and
        # outputs flow from the first segment on.
        tsum_sb = tsump.tile([P, NBT * G * H], BF16)
        for bt in range(NBT):
            for g in range(G):
                ps = pss.tile([P, H], F32)
                for hc in range(HC):
                    nc.tensor.matmul(
                        ps[:],
                        st_slice(hc, bt),
                        btw_slice(g, hc),
                        start=(hc == 0),
                        stop=(hc == HC - 1),
                    )
                c = (bt * G + g) * H
                nc.scalar.copy(tsum_sb[:, c : c + H], ps[:])

            for k in range(K):
                ot = opool.tile([P, G * H], out_dt)
                for g in range(G):
                    pd = psd.tile([P, H], F32)
                    for hc in range(HC):
                        nc.tensor.matmul(
                            pd[:],
                            xt_slice(bt, k, hc),
                            dt_sb[:, (g * HC + hc) * H : (g * HC + hc + 1) * H],
                            start=(hc == 0),
                            stop=(hc == HC - 1),
                        )
                    c = (bt * G + g) * H
                    nc.vector.tensor_add(
                        ot[:, g * H : (g + 1) * H], pd[:], tsum_sb[:, c : c + H]
                    )
                    if bt == NBT - 1 and k == K - 1:
                        # very last group: drain per-g so the final
                        # transfer after the last matmul is small
                        nc.scalar.dma_start(
                            y[bt, k, :, g * H : (g + 1) * H],
                            ot[:, g * H : (g + 1) * H],
                        )
                if not (bt == NBT - 1 and k == K - 1):
                    nc.scalar.dma_start(y[bt, k, :, :], ot[:])

    nc.compile()
    return nc


def _get_nc():
    if "nc" not in _CACHE:
        _CACHE["nc"] = _build()
    return _CACHE["nc"]


def _prep_inputs(x, A, B):
    x = np.ascontiguousarray(np.asarray(x, dtype=np.float32))
    A = np.asarray(A, dtype=np.float32)
    B = np.asarray(B, dtype=np.float32)

    # [q, k, hc, b_global] -> later repacked per-core to (k, bt, hc, b128)
    xt_full = np.ascontiguousarray(
        x.T.reshape(K, HC, P, B_TOTAL).transpose(2, 0, 1, 3)
    )
    if MM_BF16:
        import ml_dtypes

        xt_full = xt_full.astype(ml_dtypes.bfloat16)
    s_full = x.reshape(B_TOTAL, K, H).sum(axis=1, dtype=np.float32)
    st_full = np.ascontiguousarray(
        s_full.T.reshape(HC, P, B_TOTAL).transpose(1, 0, 2)
    )
    # [q, g, hc, p]
    D = A - B
    dtw = np.ascontiguousarray(
        D.reshape(G, H, HC, P).transpose(3, 0, 2, 1)
    )
    btw = np.ascontiguousarray(
        B.reshape(G, H, HC, P).transpose(3, 0, 2, 1)
    )
    if MM_BF16:
        import ml_dtypes

        st_full = st_full.astype(ml_dtypes.bfloat16)
        dtw = dtw.astype(ml_dtypes.bfloat16)
        btw = btw.astype(ml_dtypes.bfloat16)

    in_maps = []
    for c in range(NCORES):
        cols = slice(c * BS, (c + 1) * BS)
        stc = st_full[:, :, cols].reshape(P, HC, NBT, P)  # [P, hc, bt, b']
        # chunk0: for hc: [ st(bt0, hc) 128c | btw(g0, hc) 512c ]
        c0 = np.concatenate([stc[:, :, 0, :], btw[:, 0, :, :]], axis=2)
        st_rest = stc[:, :, 1:, :].transpose(0, 2, 1, 3)  # [P, bt1..3, hc, b']
        x_bt = (
            xt_full[:, :, :, cols]
            .reshape(P, K, HC, NBT, P)
            .transpose(0, 3, 1, 2, 4)  # [P, bt, k, hc, b']
        )
        packed = np.concatenate(
            [
                c0.reshape(P, HC * 640),
                st_rest.reshape(P, (NBT - 1) * HC * P),
                btw[:, 1:, :, :].reshape(P, (G - 1) * HC * H),
                dtw.reshape(P, G * HC * H),
                x_bt.reshape(P, K * HC * BS),
            ],
            axis=1,
        )
        in_maps.append({"inp": np.ascontiguousarray(packed)})
    return in_maps


def _unpack_output(res):
    outs = []
    for c in range(NCORES):
        yd = np.asarray(res.results[c]["y"]).astype(np.float32)
        # [bt, k, p, g, pp] -> [bt, p, g, k, pp]
        yc = yd.reshape(NBT, K, P, G, H).transpose(0, 2, 3, 1, 4)
        outs.append(np.ascontiguousarray(yc).reshape(BS, G * K * H))
    return np.concatenate(outs, axis=0)


def _run(x, A, B, **run_kwargs):
    in_maps = _prep_inputs(x, A, B)
    nc = _get_nc()
    res = run_bass_kernel_spmd(nc, in_maps, list(range(NCORES)), **run_kwargs)
    return _unpack_output(res), res


def kernel(x, A, B):
    out, _ = _run(x, A, B)
    return out


# revision 30
# speedup vs baseline: 1.0300x; 1.0300x over previous
"""BlockEqLinear kernel for Trainium2 (8 NeuronCores, SPMD data-parallel over batch).

Math (reference):
    x: [4096, 4096] viewed as [B=4096, K=8, H=512]
    A, B: [G=4, H, H]
    out[b, g, k, :] = x_k[b] @ (A_g - B_g)^T + S[b] @ B_g^T,  S = sum_k x_k
    returned as [B, G*K*H] = [4096, 16384]

Strategy (measured ~146 us HW time on 8 cores; PE-stream floor ~124 us):
  - Shard batch across 8 cores (512 rows each); weights replicated.
  - Host-side layout prep only (transposes/packing + the trivial S
    reduction): all inputs packed into ONE partition-major DRAM tensor
    per core so a few large column-chunk DMAs feed a single resident
    SBUF tile; contraction dim (h) is the partition dim on chip.
  - Matmul operands in bf16 (PSUM accumulates fp32): same PE rate as
    fp32r but half the input DMA and fast weight loads; measured
    back-to-back matmul interval 216 ns = the N=512 streaming floor.
  - Dummy warm-up matmuls run during the input-DMA wait so the PE HAM
    clock gate is at 8/8 (2.4 GHz) when the real stream starts.
  - Phase A: tsum[bt,g] = S-tile @ B_g^T (g-major; first group needs
    only the first 0.25 MB chunk), evicted to SBUF by ScalarE.
  - Phase B: k-outer so x^T streams just-in-time one k-slice at a time:
    for k: for bt: for g: 4 matmuls -> PSUM; DVE eviction fuses the
    +tsum add and packs 4 g-slices into one staging tile; one output
    DMA per (k, bt) on the second HWDGE queue (ScalarE).
  - Output written as bf16 (halves write traffic -> PE-bound overall),
    upcast to fp32 on host. L2 rel err ~3.2e-3, absmax ~0.5% of scale.
"""

import numpy as np

import concourse.mybir as mybir
import concourse.tile as tile
from concourse import bacc
from concourse.bass_utils import run_bass_kernel_spmd
from contextlib import ExitStack

G, K, H = 4, 8, 512
B_TOTAL = 4096
NCORES = 8
BS = B_TOTAL // NCORES  # 512 batch rows per core
P = 128                 # partition dim
HC = H // P             # 4 contraction chunks per 512-dim h
NBT = BS // P           # 4 b-tiles per core

F32 = mybir.dt.float32
F32R = mybir.dt.float32r
BF16 = mybir.dt.bfloat16

OUT_BF16 = True         # write y as bf16 on device, upcast on host
MM_BF16 = True          # all matmul operands (x^T, S^T, weights) in bf16:
                        # halves input DMA, enables fast weight load
N_WARMUP = 9            # dummy PE matmuls to warm HAM during input DMA

_CACHE = {}


def _build():
    out_dt = BF16 if OUT_BF16 else F32

    nc = bacc.Bacc(
        "TRN2", target_bir_lowering=False, debug=False, num_devices=NCORES
    )

    # All inputs packed host-side into ONE partition-major tensor so a
    # few large column-chunk DMAs (one descriptor-gen each) feed SBUF.
    # Column layout (all per-partition-row contiguous in DRAM):
    #   [0, 4096):      for hc: [ st(hc,:) 512c | btw(g0,hc,:) 512c ]
    #                   (interleaved so the first sum matmul needs only
    #                   the first 1024 cols)
    #   [4096, 10240):  btw[g1..3]   (g, hc, p)
    #   [10240, 18432): dtw          (g, hc, p)
    #   [18432, 34816): xt           (k, hc, b)
    x_dt = BF16 if MM_BF16 else F32R
    NCOL_ST = HC * BS
    NCOL_W = G * HC * H
    NCOL_X = K * HC * BS
    NCOL = NCOL_ST + 2 * NCOL_W + NCOL_X
    inp = nc.dram_tensor("inp", [P, NCOL], x_dt, kind="ExternalInput")
    # y_dev[bt, k, p, g*512 + pp] = out[bt*128 + p, g, k, pp]
    y = nc.dram_tensor("y", [NBT, K, P, G * H], out_dt, kind="ExternalOutput")

    with tile.TileContext(nc) as tc, ExitStack() as ctx:
        wpool = ctx.enter_context(tc.tile_pool(name="w", bufs=1))
        xpool = ctx.enter_context(tc.tile_pool(name="x", bufs=1))
        tsump = ctx.enter_context(tc.tile_pool(name="tsum", bufs=1))
        opool = ctx.enter_context(tc.tile_pool(name="o", bufs=8))
        psd = ctx.enter_context(tc.tile_pool(name="psd", bufs=6, space="PSUM"))
        pss = ctx.enter_context(tc.tile_pool(name="pss", bufs=2, space="PSUM"))

        # PE warm-up scratch: zeroed tile for dummy matmuls (below) that
        # run while the first input DMAs are in flight, so HAM is at
        # K=8/8 (2.4 GHz) when the real matmul stream starts.
        scratch = wpool.tile([P, H], BF16)
        nc.gpsimd.memset(scratch[:], 0.0)

        # One SBUF-resident input tile; chunked column DMAs in
        # consumption order. chunk0 is split per-hc so the first sum
        # matmul can start after only 1024 cols (0.25 MB).
        in_sb = xpool.tile([P, NCOL], x_dt)
        O_DT = NCOL_ST + NCOL_W
        O_XT = NCOL_ST + 2 * NCOL_W
        chunks = [(hc * 1024, (hc + 1) * 1024) for hc in range(HC)]
        chunks += [
            (4096, O_DT),                             # btw[g1..3]
            (O_DT, O_DT + NCOL_W // 2),               # dtw[g0..1]
            (O_XT, O_XT + HC * BS),                   # xt[k0] (needed with dtw)
            (O_DT + NCOL_W // 2, O_XT),               # dtw[g2..3]
        ]
        for k in range(1, K):
            chunks.append((O_XT + k * HC * BS, O_XT + (k + 1) * HC * BS))
        for c0, c1 in chunks:
            nc.sync.dma_start(in_sb[:, c0:c1], inp[:, c0:c1])

        def st_slice(hc, b0):
            return in_sb[:, hc * 1024 + b0 : hc * 1024 + b0 + P]

        def btw_slice(g, hc):
            if g == 0:
                c = hc * 1024 + 512
            else:
                c = 4096 + ((g - 1) * HC + hc) * H
            return in_sb[:, c : c + H]

        dt_sb = in_sb[:, O_DT : O_DT + NCOL_W]
        xt_sb = in_sb[:, O_XT : O_XT + NCOL_X]

        # Dummy warm-up matmuls (PE program order puts these before the
        # real stream; they execute during the input-DMA wait).
        warm_ps = pss.tile([P, H], F32, tag="ps")
        for i in range(N_WARMUP):
            nc.tensor.matmul(
                warm_ps[:],
                scratch[:, :P],
                scratch[:],
                start=True,
                stop=True,
            )

        # Phase A: tsum[bt, g] = S-tile @ B_g^T for all 16 (bt, g) pairs.
        # g-major so the first 16 matmuls need only st + btw[g=0].
        tsum_sb = tsump.tile([P, NBT * G * H], BF16)
        for g in range(G):
            for bt in range(NBT):
                b0 = bt * P
                ps = pss.tile([P, H], F32)
                for hc in range(HC):
                    nc.tensor.matmul(
                        ps[:],
                        st_slice(hc, b0),
                        btw_slice(g, hc),
                        start=(hc == 0),
                        stop=(hc == HC - 1),
                    )
                c = (bt * G + g) * H
                nc.scalar.copy(tsum_sb[:, c : c + H], ps[:])

        # Phase B: k-outer diag matmuls; pack 4 g-slices per (k, bt).
        for k in range(K):
            for bt in range(NBT):
                b0 = bt * P
                ot = opool.tile([P, G * H], out_dt)
                for g in range(G):
                    pd = psd.tile([P, H], F32)
                    for hc in range(HC):
                        xb = (k * HC + hc) * BS + b0
                        nc.tensor.matmul(
                            pd[:],
                            xt_sb[:, xb : xb + P],
                            dt_sb[:, (g * HC + hc) * H : (g * HC + hc + 1) * H],
                            start=(hc == 0),
                            stop=(hc == HC - 1),
                        )
                    c = (bt * G + g) * H
                    nc.vector.tensor_add(
                        ot[:, g * H : (g + 1) * H], pd[:], tsum_sb[:, c : c + H]
                    )
                    if k == K - 1:
                        # last k-group: drain per-g so the final transfer
                        # after the last matmul is small
                        nc.scalar.dma_start(
                            y[bt, k, :, g * H : (g + 1) * H],
                            ot[:, g * H : (g + 1) * H],
                        )
                if k < K - 1:
                    nc.scalar.dma_start(y[bt, k, :, :], ot[:])

    nc.compile()
    return nc


def _get_nc():
    if "nc" not in _CACHE:
        _CACHE["nc"] = _build()
    return _CACHE["nc"]


def _prep_inputs(x, A, B):
    x = np.ascontiguousarray(np.asarray(x, dtype=np.float32))
    A = np.asarray(A, dtype=np.float32)
    B = np.asarray(B, dtype=np.float32)

    # [q, k, hc, b_global]
    xt_full = np.ascontiguousarray(
        x.T.reshape(K, HC, P, B_TOTAL).transpose(2, 0, 1, 3)
    )
    if MM_BF16:
        import ml_dtypes

        xt_full = xt_full.astype(ml_dtypes.bfloat16)
    s_full = x.reshape(B_TOTAL, K, H).sum(axis=1, dtype=np.float32)
    st_full = np.ascontiguousarray(
        s_full.T.reshape(HC, P, B_TOTAL).transpose(1, 0, 2)
    )
    # [q, g, hc, p]
    D = A - B
    dtw = np.ascontiguousarray(
        D.reshape(G, H, HC, P).transpose(3, 0, 2, 1)
    )
    btw = np.ascontiguousarray(
        B.reshape(G, H, HC, P).transpose(3, 0, 2, 1)
    )
    if MM_BF16:
        import ml_dtypes

        st_full = st_full.astype(ml_dtypes.bfloat16)
        dtw = dtw.astype(ml_dtypes.bfloat16)
        btw = btw.astype(ml_dtypes.bfloat16)

    in_maps = []
    for c in range(NCORES):
        cols = slice(c * BS, (c + 1) * BS)
        stc = st_full[:, :, cols]  # [P, HC, BS]
        # chunk0: for hc: [ st(hc) | btw(g0, hc) ]
        c0 = np.concatenate([stc, btw[:, 0, :, :]], axis=2)  # [P, HC, 1024]
        packed = np.concatenate(
            [
                c0.reshape(P, HC * 1024),
                btw[:, 1:, :, :].reshape(P, (G - 1) * HC * H),
                dtw.reshape(P, G * HC * H),
                xt_full[:, :, :, cols].reshape(P, K * HC * BS),
            ],
            axis=1,
        )
        in_maps.append({"inp": np.ascontiguousarray(packed)})
    return in_maps


def _unpack_output(res):
    outs = []
    for c in range(NCORES):
        yd = np.asarray(res.results[c]["y"]).astype(np.float32)
        # [bt, k, p, g, pp] -> [bt, p, g, k, pp]
        yc = yd.reshape(NBT, K, P, G, H).transpose(0, 2, 3, 1, 4)
        outs.append(np.ascontiguousarray(yc).reshape(BS, G * K * H))
    return np.concatenate(outs, axis=0)


def _run(x, A, B, **run_kwargs):
    in_maps = _prep_inputs(x, A, B)
    nc = _get_nc()
    res = run_bass_kernel_spmd(nc, in_maps, list(range(NCORES)), **run_kwargs)
    return _unpack_output(res), res


def kernel(x, A, B):
    out, _ = _run(x, A, B)
    return out


# revision 31
# speedup vs baseline: 1.0302x; 1.0002x over previous
"""BlockEqLinear kernel for Trainium2 (8 NeuronCores, SPMD data-parallel over batch).

Math (reference):
    x: [4096, 4096] viewed as [B=4096, K=8, H=512]
    A, B: [G=4, H, H]
    out[b, g, k, :] = x_k[b] @ (A_g - B_g)^T + S[b] @ B_g^T,  S = sum_k x_k
    returned as [B, G*K*H] = [4096, 16384]

Strategy (measured ~146 us HW time on 8 cores; PE-stream floor ~124 us):
  - Shard batch across 8 cores (512 rows each); weights replicated.
  - Host-side layout prep only (transposes/packing + the trivial S
    reduction): all inputs packed into ONE partition-major DRAM tensor
    per core so a few large column-chunk DMAs feed a single resident
    SBUF tile; contraction dim (h) is the partition dim on chip.
  - Matmul operands in bf16 (PSUM accumulates fp32): same PE rate as
    fp32r but half the input DMA and fast weight loads; measured
    back-to-back matmul interval 216 ns = the N=512 streaming floor.
  - Dummy warm-up matmuls run during the input-DMA wait so the PE HAM
    clock gate is at 8/8 (2.4 GHz) when the real stream starts.
  - Phase A: tsum[bt,g] = S-tile @ B_g^T (g-major; first group needs
    only the first 0.25 MB chunk), evicted to SBUF by ScalarE.
  - Phase B: k-outer so x^T streams just-in-time one k-slice at a time:
    for k: for bt: for g: 4 matmuls -> PSUM; DVE eviction fuses the
    +tsum add and packs 4 g-slices into one staging tile; one output
    DMA per (k, bt) on the second HWDGE queue (ScalarE).
  - Output written as bf16 (halves write traffic -> PE-bound overall),
    upcast to fp32 on host. L2 rel err ~3.2e-3, absmax ~0.5% of scale.
"""

import numpy as np

import concourse.mybir as mybir
import concourse.tile as tile
from concourse import bacc
from concourse.bass_utils import run_bass_kernel_spmd
from contextlib import ExitStack

G, K, H = 4, 8, 512
B_TOTAL = 4096
NCORES = 8
BS = B_TOTAL // NCORES  # 512 batch rows per core
P = 128                 # partition dim
HC = H // P             # 4 contraction chunks per 512-dim h
NBT = BS // P           # 4 b-tiles per core

F32 = mybir.dt.float32
F32R = mybir.dt.float32r
BF16 = mybir.dt.bfloat16

OUT_BF16 = True         # write y as bf16 on device, upcast on host
MM_BF16 = True          # all matmul operands (x^T, S^T, weights) in bf16:
                        # halves input DMA, enables fast weight load
N_WARMUP = 9            # dummy PE matmuls to warm HAM during input DMA

_CACHE = {}


def _build():
    out_dt = BF16 if OUT_BF16 else F32

    nc = bacc.Bacc(
        "TRN2", target_bir_lowering=False, debug=False, num_devices=NCORES
    )

    # All inputs packed host-side into ONE partition-major tensor so a
    # few large column-chunk DMAs (one descriptor-gen each) feed SBUF.
    # Column layout (all per-partition-row contiguous in DRAM):
    #   [0, 4096):      for hc: [ st(hc,:) 512c | btw(g0,hc,:) 512c ]
    #                   (interleaved so the first sum matmul needs only
    #                   the first 1024 cols)
    #   [4096, 10240):  btw[g1..3]   (g, hc, p)
    #   [10240, 18432): dtw          (g, hc, p)
    #   [18432, 34816): xt           (k, hc, b)
    x_dt = BF16 if MM_BF16 else F32R
    NCOL_ST = HC * BS
    NCOL_W = G * HC * H
    NCOL_X = K * HC * BS
    NCOL = NCOL_ST + 2 * NCOL_W + NCOL_X
    inp = nc.dram_tensor("inp", [P, NCOL], x_dt, kind="ExternalInput")
    # y_dev[bt, k, p, g*512 + pp] = out[bt*128 + p, g, k, pp]
    y = nc.dram_tensor("y", [NBT, K, P, G * H], out_dt, kind="ExternalOutput")

    with tile.TileContext(nc) as tc, ExitStack() as ctx:
        wpool = ctx.enter_context(tc.tile_pool(name="w", bufs=1))
        xpool = ctx.enter_context(tc.tile_pool(name="x", bufs=1))
        tsump = ctx.enter_context(tc.tile_pool(name="tsum", bufs=1))
        opool = ctx.enter_context(tc.tile_pool(name="o", bufs=8))
        psd = ctx.enter_context(tc.tile_pool(name="psd", bufs=6, space="PSUM"))
        pss = ctx.enter_context(tc.tile_pool(name="pss", bufs=2, space="PSUM"))

        # PE warm-up scratch: zeroed tile for dummy matmuls (below) that
        # run while the first input DMAs are in flight, so HAM is at
        # K=8/8 (2.4 GHz) when the real matmul stream starts.
        scratch = wpool.tile([P, H], BF16)
        nc.gpsimd.memset(scratch[:], 0.0)

        # One SBUF-resident input tile; chunked column DMAs in
        # consumption order. chunk0 is split per-hc so the first sum
        # matmul can start after only 1024 cols (0.25 MB).
        in_sb = xpool.tile([P, NCOL], x_dt)
        O_DT = NCOL_ST + NCOL_W
        O_XT = NCOL_ST + 2 * NCOL_W
        chunks = [(hc * 1024, (hc + 1) * 1024) for hc in range(HC)]
        chunks += [
            (4096, O_DT),                             # btw[g1..3]
            (O_DT, O_DT + NCOL_W // 2),               # dtw[g0..1]
            (O_XT, O_XT + HC * BS),                   # xt[k0] (needed with dtw)
            (O_DT + NCOL_W // 2, O_XT),               # dtw[g2..3]
        ]
        for k in range(1, K):
            chunks.append((O_XT + k * HC * BS, O_XT + (k + 1) * HC * BS))
        for c0, c1 in chunks:
            nc.sync.dma_start(in_sb[:, c0:c1], inp[:, c0:c1])

        def st_slice(hc, b0):
            return in_sb[:, hc * 1024 + b0 : hc * 1024 + b0 + P]

        def btw_slice(g, hc):
            if g == 0:
                c = hc * 1024 + 512
            else:
                c = 4096 + ((g - 1) * HC + hc) * H
            return in_sb[:, c : c + H]

        dt_sb = in_sb[:, O_DT : O_DT + NCOL_W]
        xt_sb = in_sb[:, O_XT : O_XT + NCOL_X]

        # Dummy warm-up matmuls (PE program order puts these before the
        # real stream; they execute during the input-DMA wait).
        warm_ps = pss.tile([P, H], F32, tag="ps")
        for i in range(N_WARMUP):
            nc.tensor.matmul(
                warm_ps[:],
                scratch[:, :P],
                scratch[:],
                start=True,
                stop=True,
            )

        # Phase A: tsum[bt, g] = S-tile @ B_g^T for all 16 (bt, g) pairs.
        # g-major so the first 16 matmuls need only st + btw[g=0].
        tsum_sb = tsump.tile([P, NBT * G * H], F32)
        for g in range(G):
            for bt in range(NBT):
                b0 = bt * P
                ps = pss.tile([P, H], F32)
                for hc in range(HC):
                    nc.tensor.matmul(
                        ps[:],
                        st_slice(hc, b0),
                        btw_slice(g, hc),
                        start=(hc == 0),
                        stop=(hc == HC - 1),
                    )
                c = (bt * G + g) * H
                nc.scalar.copy(tsum_sb[:, c : c + H], ps[:])

        # Phase B: k-outer diag matmuls; pack 4 g-slices per (k, bt).
        for k in range(K):
            for bt in range(NBT):
                b0 = bt * P
                ot = opool.tile([P, G * H], out_dt)
                for g in range(G):
                    pd = psd.tile([P, H], F32)
                    for hc in range(HC):
                        xb = (k * HC + hc) * BS + b0
                        nc.tensor.matmul(
                            pd[:],
                            xt_sb[:, xb : xb + P],
                            dt_sb[:, (g * HC + hc) * H : (g * HC + hc + 1) * H],
                            start=(hc == 0),
                            stop=(hc == HC - 1),
                        )
                    c = (bt * G + g) * H
                    nc.vector.tensor_add(
                        ot[:, g * H : (g + 1) * H], pd[:], tsum_sb[:, c : c + H]
                    )
                    if k == K - 1:
                        # last k-group: drain per-g so the final transfer
                        # after the last matmul is small
                        nc.scalar.dma_start(
                            y[bt, k, :, g * H : (g + 1) * H],
                            ot[:, g * H : (g + 1) * H],
                        )
                if k < K - 1:
                    nc.scalar.dma_start(y[bt, k, :, :], ot[:])

    nc.compile()
    return nc


def _get_nc():
    if "nc" not in _CACHE:
        _CACHE["nc"] = _build()
    return _CACHE["nc"]


def _prep_inputs(x, A, B):
    x = np.ascontiguousarray(np.asarray(x, dtype=np.float32))
    A = np.asarray(A, dtype=np.float32)
    B = np.asarray(B, dtype=np.float32)

    # [q, k, hc, b_global]
    xt_full = np.ascontiguousarray(
        x.T.reshape(K, HC, P, B_TOTAL).transpose(2, 0, 1, 3)
    )
    if MM_BF16:
        import ml_dtypes

        xt_full = xt_full.astype(ml_dtypes.bfloat16)
    s_full = x.reshape(B_TOTAL, K, H).sum(axis=1, dtype=np.float32)
    st_full = np.ascontiguousarray(
        s_full.T.reshape(HC, P, B_TOTAL).transpose(1, 0, 2)
    )
    # [q, g, hc, p]
    D = A - B
    dtw = np.ascontiguousarray(
        D.reshape(G, H, HC, P).transpose(3, 0, 2, 1)
    )
    btw = np.ascontiguousarray(
        B.reshape(G, H, HC, P).transpose(3, 0, 2, 1)
    )
    if MM_BF16:
        import ml_dtypes

        st_full = st_full.astype(ml_dtypes.bfloat16)
        dtw = dtw.astype(ml_dtypes.bfloat16)
        btw = btw.astype(ml_dtypes.bfloat16)

    in_maps = []
    for c in range(NCORES):
        cols = slice(c * BS, (c + 1) * BS)
        stc = st_full[:, :, cols]  # [P, HC, BS]
        # chunk0: for hc: [ st(hc) | btw(g0, hc) ]
        c0 = np.concatenate([stc, btw[:, 0, :, :]], axis=2)  # [P, HC, 1024]
        packed = np.concatenate(
            [
                c0.reshape(P, HC * 1024),
                btw[:, 1:, :, :].reshape(P, (G - 1) * HC * H),
                dtw.reshape(P, G * HC * H),
                xt_full[:, :, :, cols].reshape(P, K * HC * BS),
            ],
            axis=1,
        )
        in_maps.append({"inp": np.ascontiguousarray(packed)})
    return in_maps


def _unpack_output(res):
    outs = []
    for c in range(NCORES):
        yd = np.asarray(res.results[c]["y"]).astype(np.float32)
        # [bt, k, p, g, pp] -> [bt, p, g, k, pp]
        yc = yd.reshape(NBT, K, P, G, H).transpose(0, 2, 3, 1, 4)
        outs.append(np.ascontiguousarray(yc).reshape(BS, G * K * H))
    return np.concatenate(outs, axis=0)


def _run(x, A, B, **run_kwargs):
    in_maps = _prep_inputs(x, A, B)
    nc = _get_nc()
    res = run_bass_kernel_spmd(nc, in_maps, list(range(NCORES)), **run_kwargs)
    return _unpack_output(res), res


def kernel(x, A, B):
    out, _ = _run(x, A, B)
    return out
